# revision 25
# baseline (speedup 1.0000x reference)
"""Trainium2 Bass kernel for grouped cumulative-sim causal attention.

Reference computation (B=2, N=2048, G=4 groups, H=8 heads, DH=64):
  q/k/v = 1x1-conv projections of x [B, 2048, N]
  sim[b,g,h] = cumsum_over_g( (SCALE*q) @ k^T )   (the group-cumsum)
  out = softmax(causal(sim)) @ v ; y = Wout @ out + b_out

Sharding: one head h per NeuronCore (8 cores), both batches and all 4
groups local to the core (the cumsum couples g only). Each core computes
a partial y (its head's 256-channel contribution through Wout); the host
sums the 8 partials and adds b_out.

Device-side layout (everything transposed so the softmax j-axis lands on
PSUM partitions and attn comes out ready for the AV matmul):
  q,k  [dh=64(part,2 groups/tile), gpair, i/j]   (Wq pre-scaled by SCALE)
  sim_T[j(part), i]  accumulated in PSUM across g  -> group cumsum is free
  causal mask   pre-added into PSUM via identity-matmul of a -1e30 tile
  exp           ScalarE PSUM->SBUF
  AV            lhsT = v^T tile augmented with a ones column -> row 64 of
                the PSUM output accumulates the softmax denominator
  1/s broadcast K=1 matmul of the reciprocal row
  y             Wout^T-slice matmul, partials DMA'd out
Matmuls use float32r (full-rate fp32 on the PE at moving-dim>=256).
"""

import numpy as np

B, N = 2, 2048
G, H, DH = 4, 8, 64
CIN = 2048            # input channels  (= DIM*G)
PH = G * DH           # 256 inner channels per head
SCALE = DH ** -0.5
P = 128
FB = 512              # i-block width (fp32 moving-dim max)
NB = N // FB          # 4 i-blocks
CT = CIN // P         # 16 contraction tiles
JT = N // P           # 16 j-tiles

_cache = {}


def _build_program():
    import concourse.bass as bass  # noqa: F401
    import concourse.tile as tile
    from concourse import bacc, mybir

    f32 = mybir.dt.float32
    f32r = mybir.dt.float32r
    bf16 = mybir.dt.bfloat16
    Exp = mybir.ActivationFunctionType.Exp

    # Bacc (not raw Bass): its compile() splits multi-wait matmuls
    # (move_matmul_waits_to_ldweights / generate_event_semaphores) —
    # the S3_LW matmul instruction has a single hardware wait slot.
    nc = bacc.Bacc(None, target_bir_lowering=False)
    x_d = nc.dram_tensor("x", [B, CIN, N], f32r, kind="ExternalInput")
    wq_d = nc.dram_tensor("wqT", [CIN, PH], f32r, kind="ExternalInput")
    wk_d = nc.dram_tensor("wkT", [CIN, PH], f32r, kind="ExternalInput")
    wv_d = nc.dram_tensor("wvT", [CIN, PH], f32r, kind="ExternalInput")
    wo_d = nc.dram_tensor("woT", [PH, CIN], f32r, kind="ExternalInput")
    tri_d = nc.dram_tensor("tri", [4, P, FB], bf16, kind="ExternalInput")
    id_d = nc.dram_tensor("ident", [P, P], bf16, kind="ExternalInput")
    on_d = nc.dram_tensor("onesr", [1, 64], f32r, kind="ExternalInput")
    vo_d = nc.dram_tensor("vones", [P, JT, 4], f32r, kind="ExternalInput")
    y_d = nc.dram_tensor("y", [B, CIN, N], f32, kind="ExternalOutput")

    with tile.TileContext(nc) as tc:
        from contextlib import ExitStack

        with ExitStack() as ctx:
            consts = ctx.enter_context(tc.tile_pool(name="consts", bufs=1))
            big = ctx.enter_context(tc.tile_pool(name="big", bufs=1))
            xp = ctx.enter_context(tc.tile_pool(name="xp", bufs=18))
            atp = ctx.enter_context(tc.tile_pool(name="atp", bufs=5))
            avsp = ctx.enter_context(tc.tile_pool(name="avsp", bufs=1))
            rcp = ctx.enter_context(tc.tile_pool(name="rcp", bufs=2))
            bcp = ctx.enter_context(tc.tile_pool(name="bcp", bufs=2))
            ysp = ctx.enter_context(tc.tile_pool(name="ysp", bufs=3))
            simp = ctx.enter_context(tc.tile_pool(name="simp", bufs=2, space="PSUM"))
            avp = ctx.enter_context(tc.tile_pool(name="avp", bufs=4, space="PSUM"))
            gp = ctx.enter_context(tc.tile_pool(name="gp", bufs=2, space="PSUM"))

            # ---- static tensors ----
            wq_sb = consts.tile([P, CT, PH], f32r)
            wk_sb = consts.tile([P, CT, PH], f32r)
            wv_sb = consts.tile([P, CT, PH], f32r)
            wo_sb = consts.tile([P, 2, CIN], f32r)
            nc.sync.dma_start(wq_sb, wq_d[:, :].rearrange("(co ci) m -> ci co m", ci=P))
            nc.sync.dma_start(wk_sb, wk_d[:, :].rearrange("(co ci) m -> ci co m", ci=P))
            nc.sync.dma_start(wv_sb, wv_d[:, :].rearrange("(co ci) m -> ci co m", ci=P))
            nc.sync.dma_start(wo_sb, wo_d[:, :].rearrange("(ko ki) m -> ki ko m", ki=P))
            tri_sb = consts.tile([P, 4, FB], bf16)
            nc.sync.dma_start(tri_sb, tri_d[:, :, :].rearrange("t p f -> p t f"))
            id_sb = consts.tile([P, P], bf16)
            nc.sync.dma_start(id_sb, id_d[:, :])
            # ones row at partition 64 (matmul lhsT/rhs must share base
            # partition; the softmax denominator lives at partition 64)
            on_sb = consts.tile([65, 64], f32r)
            nc.sync.dma_start(on_sb[64:65, :], on_d[:, :])

            for b in range(B):
                q_sb = big.tile([P, 2, N], f32r, tag="q")
                k_sb = big.tile([P, 2, N], f32r, tag="k")
                v_sb = big.tile([P, JT, 4 * 65], f32r, tag="v")
                yin = big.tile([P, 2, N], f32r, tag="yin")
                # ones column per group (the softmax-denominator row of AV)
                nc.sync.dma_start(
                    v_sb.rearrange("p t (g c) -> p t g c", g=4)[:, :, :, 64:65],
                    vo_d[:, :, :].rearrange("p t g -> p t g ()"),
                )

                for ib in range(NB):
                    isl = slice(ib * FB, (ib + 1) * FB)
                    # ---------- phase 1: projections for this i-block ----------
                    xts = []
                    for ct in range(CT):
                        xt = xp.tile([P, FB], f32r, tag="xt")
                        nc.sync.dma_start(
                            xt, x_d[b, ct * P:(ct + 1) * P, isl]
                        )
                        xts.append(xt)
                    # q,k : 2 row-tiles each (rows = [g even | g odd] x 64)
                    for dest, wsb in ((q_sb, wq_sb), (k_sb, wk_sb)):
                        for m in range(2):
                            ps = gp.tile([P, FB], f32, tag="gp")
                            for ct in range(CT):
                                nc.tensor.matmul(
                                    ps,
                                    wsb[:, ct, m * P:(m + 1) * P],
                                    xts[ct],
                                    start=(ct == 0),
                                    stop=(ct == CT - 1),
                                )
                            nc.vector.tensor_copy(dest[:, m, isl], ps)
                    # v^T : 4 j-tiles (j == i positions of this block)
                    for jj in range(4):
                        jt = ib * 4 + jj
                        ps = gp.tile([P, PH], f32, tag="gp")
                        for ct in range(CT):
                            nc.tensor.matmul(
                                ps,
                                xts[ct][:, jj * P:(jj + 1) * P],
                                wv_sb[:, ct, :],
                                start=(ct == 0),
                                stop=(ct == CT - 1),
                            )
                        nc.vector.tensor_copy(
                            v_sb[:, jt, :].rearrange("p (g c) -> p g c", g=4)[:, :, 0:64],
                            ps.rearrange("p (g c) -> p g c", g=4),
                        )

                    # ---------- phase 2: attention for this i-block ----------
                    jmax = 4 * (ib + 1)
                    avs_t = [
                        avp.tile([65, FB], f32, tag="av", name=f"av{g}")
                        for g in range(4)
                    ]
                    for jt in range(jmax):
                        sim = simp.tile([P, FB], f32, tag="sim")
                        diag = jt >= 4 * ib
                        if diag:
                            # causal mask pre-added into PSUM (identity matmul)
                            nc.tensor.matmul(
                                sim, id_sb, tri_sb[:, jt - 4 * ib, :],
                                start=True, stop=True,
                            )
                        for g in range(4):
                            po = (g % 2) * 64
                            # stop=True after each group so the interleaved
                            # Exp read of the running cumsum is legal; stop is
                            # a no-op on hardware and accumulation continues
                            # (start=False keeps has_written semantics).
                            nc.tensor.matmul(
                                sim,
                                k_sb[po:po + 64, g // 2, jt * P:(jt + 1) * P],
                                q_sb[po:po + 64, g // 2, isl],
                                start=(g == 0 and not diag),
                                stop=True,
                                skip_group_check=(g > 0 or diag),
                            )
                            at = atp.tile([P, FB], f32r, tag="at")
                            nc.scalar.activation(at, sim, Exp)
                            nc.tensor.matmul(
                                avs_t[g],
                                v_sb[:, jt, g * 65:(g + 1) * 65],
                                at,
                                start=(jt == 0),
                                stop=(jt == jmax - 1),
                            )
                    # normalize: rows 0..63 are e@v, row 64 is the denominator
                    avs = avsp.tile([64, 4, FB], f32r, tag="avs")
                    for g in range(4):
                        rc = rcp.tile([65, FB], f32r, tag="rc")
                        with nc.allow_low_precision(
                            reason="1/s rounded to f32r for the broadcast matmul"
                        ):
                            nc.vector.reciprocal(rc[64:65, :], avs_t[g][64:65, :])
                        bcps = gp.tile([64, FB], f32, tag="gp")
                        nc.tensor.matmul(
                            bcps, on_sb[64:65, :], rc[64:65, :],
                            start=True, stop=True,
                        )
                        bc = bcp.tile([64, FB], f32, tag="bc")
                        nc.vector.tensor_copy(bc, bcps)
                        nc.vector.tensor_mul(avs[:, g, :], avs_t[g][0:64, :], bc)
                    # partition repack [64,(g)] -> [128,(gpair)] for the y matmul
                    nc.sync.dma_start(
                        yin[0:64, :, isl],
                        avs.rearrange("p (gp two) f -> p two gp f", two=2)[:, 0],
                    )
                    nc.sync.dma_start(
                        yin[64:128, :, isl],
                        avs.rearrange("p (gp two) f -> p two gp f", two=2)[:, 1],
                    )
                    # ---------- phase 3: y partial for this i-block ----------
                    for ot in range(CT):
                        yp = gp.tile([P, FB], f32, tag="gp")
                        for kc in range(2):
                            nc.tensor.matmul(
                                yp,
                                wo_sb[:, kc, ot * P:(ot + 1) * P],
                                yin[:, kc, isl],
                                start=(kc == 0),
                                stop=(kc == 1),
                            )
                        ys = ysp.tile([P, FB], f32, tag="ys")
                        nc.vector.tensor_copy(ys, yp)
                        nc.sync.dma_start(y_d[b, ot * P:(ot + 1) * P, isl], ys)
    if not nc.is_finalized():
        nc.finalize()
    return nc


def _host_inputs(x, Wq, Wkv, Wout):
    """Per-core input maps (head h on core h)."""
    import ml_dtypes

    tri = np.zeros((4, P, FB), np.float32)
    jj = np.arange(P)[:, None]
    ii = np.arange(FB)[None, :]
    for t in range(4):
        tri[t] = np.where(jj + t * P > ii, -1e30, 0.0)
    tri = tri.astype(ml_dtypes.bfloat16)
    ident = np.eye(P, dtype=np.float32).astype(ml_dtypes.bfloat16)

    x = np.ascontiguousarray(x, dtype=np.float32)
    Wk, Wv = Wkv[:CIN], Wkv[CIN:]
    in_maps = []
    for h in range(H):
        rows = (np.arange(G)[:, None] * (H * DH) + h * DH
                + np.arange(DH)[None, :]).reshape(-1)          # (g d) order
        in_maps.append({
            "x": x,
            "wqT": np.ascontiguousarray((Wq[rows] * SCALE).T, np.float32),
            "wkT": np.ascontiguousarray(Wk[rows].T, np.float32),
            "wvT": np.ascontiguousarray(Wv[rows].T, np.float32),
            "woT": np.ascontiguousarray(Wout[:, rows].T, np.float32),
            "tri": tri,
            "ident": ident,
            "onesr": np.ones((1, 64), np.float32),
            "vones": np.ones((P, JT, 4), np.float32),
        })
    return in_maps


def _install_profile_hook():
    """Register the NTFF profile hook that the image's antenv lacks, and
    keep profile artifacts local (no bucket upload)."""
    import sys
    import types
    import ctypes
    import contextlib

    if "antenv.axon_hooks" in sys.modules:
        return
    so_path = "/opt/axon/libaxon_pjrt.so"
    lib = ctypes.CDLL(so_path)
    if not hasattr(lib, "axon_start_nrt_profile"):
        raise RuntimeError("libaxon_pjrt.so lacks profiling symbols")
    lib.axon_start_nrt_profile.argtypes = [
        ctypes.POINTER(ctypes.c_int64), ctypes.c_size_t,
    ]
    lib.axon_start_nrt_profile.restype = ctypes.c_int64
    lib.axon_stop_nrt_profile.argtypes = [ctypes.c_char_p]
    lib.axon_stop_nrt_profile.restype = ctypes.c_int64

    @contextlib.contextmanager
    def _hook(output_dir, device_ids):
        import jax
        jax.devices()
        if device_ids:
            ids = (ctypes.c_int64 * len(device_ids))(*device_ids)
            rc = lib.axon_start_nrt_profile(ids, len(device_ids))
        else:
            rc = lib.axon_start_nrt_profile(None, 0)
        if rc != 0:
            raise RuntimeError(f"axon_start_nrt_profile rc={rc}")
        try:
            yield
        finally:
            n = lib.axon_stop_nrt_profile(str(output_dir).encode())
            print(f"profile: {n} file(s) written to {output_dir}")

    mod = types.ModuleType("antenv.axon_hooks")
    mod.get_axon_ntff_profile_hook = lambda: _hook
    mod.set_axon_ntff_profile_hook = lambda h: None
    sys.modules["antenv.axon_hooks"] = mod

    import concourse.bass_utils as bu
    bu.upload_artifacts = lambda tmpdir: tmpdir


def kernel(x, Wq, Wkv, Wout, b_out, _profile=False):
    import sys
    if "/opt/trn_rl_repo" not in sys.path:
        sys.path.insert(0, "/opt/trn_rl_repo")
    from concourse.bass_utils import run_bass_kernel_spmd
    if _profile:
        _install_profile_hook()

    if "nc" not in _cache:
        _cache["nc"] = _build_program()
    nc = _cache["nc"]
    in_maps = _host_inputs(
        np.asarray(x), np.asarray(Wq), np.asarray(Wkv), np.asarray(Wout)
    )
    kwargs = {}
    if _profile:
        import tempfile
        kwargs["tmpdir"] = tempfile.mkdtemp(prefix="bass_prof_")
    res = run_bass_kernel_spmd(
        nc, in_maps, core_ids=list(range(H)), trace=_profile, **kwargs
    )
    y = np.zeros((B, CIN, N), np.float64)
    for rmap in res.results:
        y += rmap["y"].astype(np.float64)
    y = y.astype(np.float32) + np.asarray(b_out)[None, :, None]
    if _profile:
        _cache["last_exec_time_ns"] = res.exec_time_ns
        _cache["last_profile_dir"] = kwargs.get("tmpdir")
        _cache["last_results"] = res
    return y


# revision 26
# speedup vs baseline: 1.1551x; 1.1551x over previous
"""Trainium2 Bass kernel for grouped cumulative-sim causal attention.

Reference computation (B=2, N=2048, G=4 groups, H=8 heads, DH=64):
  q/k/v = 1x1-conv projections of x [B, 2048, N]
  sim[b,g,h] = cumsum_over_g( (SCALE*q) @ k^T )   (the group-cumsum)
  out = softmax(causal(sim)) @ v ; y = Wout @ out + b_out

Sharding: one head h per NeuronCore (8 cores), both batches and all 4
groups local to the core (the cumsum couples g only). Each core computes
a partial y (its head's 256-channel contribution through Wout); the host
sums the 8 partials and adds b_out.

Device-side layout (everything transposed so the softmax j-axis lands on
PSUM partitions and attn comes out ready for the AV matmul):
  q,k  [dh=64(part,2 groups/tile), gpair, i/j]   (Wq pre-scaled by SCALE)
  sim_T[j(part), i]  accumulated in PSUM across g  -> group cumsum is free
  causal mask   pre-added into PSUM via identity-matmul of a -1e30 tile
  exp           ScalarE PSUM->SBUF
  AV            lhsT = v^T tile augmented with a ones column -> row 64 of
                the PSUM output accumulates the softmax denominator
  1/s broadcast K=1 matmul of the reciprocal row
  y             Wout^T-slice matmul, partials DMA'd out
Matmuls use float32r (full-rate fp32 on the PE at moving-dim>=256).
"""

import numpy as np

B, N = 2, 2048
G, H, DH = 4, 8, 64
CIN = 2048            # input channels  (= DIM*G)
PH = G * DH           # 256 inner channels per head
SCALE = DH ** -0.5
P = 128
FB = 512              # i-block width (fp32 moving-dim max)
NB = N // FB          # 4 i-blocks
CT = CIN // P         # 16 contraction tiles
JT = N // P           # 16 j-tiles

_cache = {}


def _build_program():
    import concourse.bass as bass  # noqa: F401
    import concourse.tile as tile
    from concourse import bacc, mybir

    f32 = mybir.dt.float32
    f32r = mybir.dt.float32r
    bf16 = mybir.dt.bfloat16
    Exp = mybir.ActivationFunctionType.Exp

    # Bacc (not raw Bass): its compile() splits multi-wait matmuls
    # (move_matmul_waits_to_ldweights / generate_event_semaphores) —
    # the S3_LW matmul instruction has a single hardware wait slot.
    nc = bacc.Bacc(None, target_bir_lowering=False)
    x_d = nc.dram_tensor("x", [B, CIN, N], f32r, kind="ExternalInput")
    wq_d = nc.dram_tensor("wqT", [CIN, PH], f32r, kind="ExternalInput")
    wk_d = nc.dram_tensor("wkT", [CIN, PH], f32r, kind="ExternalInput")
    wv_d = nc.dram_tensor("wvT", [CIN, PH], f32r, kind="ExternalInput")
    wo_d = nc.dram_tensor("woT", [PH, CIN], f32r, kind="ExternalInput")
    tri_d = nc.dram_tensor("tri", [4, P, FB], bf16, kind="ExternalInput")
    id_d = nc.dram_tensor("ident", [P, P], bf16, kind="ExternalInput")
    on_d = nc.dram_tensor("onesr", [1, 64], f32r, kind="ExternalInput")
    vo_d = nc.dram_tensor("vones", [P, JT, 4], f32r, kind="ExternalInput")
    y_d = nc.dram_tensor("y", [B, CIN, N], f32, kind="ExternalOutput")

    with tile.TileContext(nc) as tc:
        from contextlib import ExitStack

        with ExitStack() as ctx:
            consts = ctx.enter_context(tc.tile_pool(name="consts", bufs=1))
            big = ctx.enter_context(tc.tile_pool(name="big", bufs=1))
            xp = ctx.enter_context(tc.tile_pool(name="xp", bufs=18))
            atp = ctx.enter_context(tc.tile_pool(name="atp", bufs=5))
            avsp = ctx.enter_context(tc.tile_pool(name="avsp", bufs=1))
            rcp = ctx.enter_context(tc.tile_pool(name="rcp", bufs=2))
            bcp = ctx.enter_context(tc.tile_pool(name="bcp", bufs=2))
            ysp = ctx.enter_context(tc.tile_pool(name="ysp", bufs=3))
            simp = ctx.enter_context(tc.tile_pool(name="simp", bufs=2, space="PSUM"))
            avp = ctx.enter_context(tc.tile_pool(name="avp", bufs=4, space="PSUM"))
            gp = ctx.enter_context(tc.tile_pool(name="gp", bufs=2, space="PSUM"))

            # ---- static tensors ----
            wq_sb = consts.tile([P, CT, PH], f32r)
            wk_sb = consts.tile([P, CT, PH], f32r)
            wv_sb = consts.tile([P, CT, PH], f32r)
            wo_sb = consts.tile([P, 2, CIN], f32r)
            nc.sync.dma_start(wq_sb, wq_d[:, :].rearrange("(co ci) m -> ci co m", ci=P))
            nc.sync.dma_start(wk_sb, wk_d[:, :].rearrange("(co ci) m -> ci co m", ci=P))
            nc.sync.dma_start(wv_sb, wv_d[:, :].rearrange("(co ci) m -> ci co m", ci=P))
            nc.sync.dma_start(wo_sb, wo_d[:, :].rearrange("(ko ki) m -> ki ko m", ki=P))
            tri_sb = consts.tile([P, 4, FB], bf16)
            nc.sync.dma_start(tri_sb, tri_d[:, :, :].rearrange("t p f -> p t f"))
            id_sb = consts.tile([P, P], bf16)
            nc.sync.dma_start(id_sb, id_d[:, :])
            # ones row at partition 64 (matmul lhsT/rhs must share base
            # partition; the softmax denominator lives at partition 64)
            on_sb = consts.tile([65, 64], f32r)
            nc.sync.dma_start(on_sb[64:65, :], on_d[:, :])

            # per-batch state, allocated lazily by phase1(step) so the
            # software pipeline below controls allocation order
            state = {}

            def get_state(b):
                if b not in state:
                    q_sb = big.tile([P, 2, N], f32r, tag="q", name=f"q{b}")
                    k_sb = big.tile([P, 2, N], f32r, tag="k", name=f"k{b}")
                    v_sb = big.tile([P, JT, 4 * 65], f32r, tag="v", name=f"v{b}")
                    yin = big.tile([P, 2, N], f32r, tag="yin", name=f"yin{b}")
                    # ones column per group (softmax-denominator row of AV)
                    nc.sync.dma_start(
                        v_sb.rearrange("p t (g c) -> p t g c", g=4)[:, :, :, 64:65],
                        vo_d[:, :, :].rearrange("p t g -> p t g ()"),
                    )
                    state[b] = (q_sb, k_sb, v_sb, yin)
                return state[b]

            def phase1(b, ib):
                q_sb, k_sb, v_sb, _ = get_state(b)
                isl = slice(ib * FB, (ib + 1) * FB)
                xts = []
                for ct in range(CT):
                    xt = xp.tile([P, FB], f32r, tag="xt", name=f"xt{ct}")
                    nc.sync.dma_start(xt, x_d[b, ct * P:(ct + 1) * P, isl])
                    xts.append(xt)
                # q,k : 2 row-tiles each (rows = [g even | g odd] x 64)
                for dest, wsb in ((q_sb, wq_sb), (k_sb, wk_sb)):
                    for m in range(2):
                        ps = gp.tile([P, FB], f32, tag="gp", name="qkps")
                        for ct in range(CT):
                            nc.tensor.matmul(
                                ps, wsb[:, ct, m * P:(m + 1) * P], xts[ct],
                                start=(ct == 0), stop=(ct == CT - 1),
                            )
                        nc.vector.tensor_copy(dest[:, m, isl], ps)
                # v^T : 4 j-tiles (j == i positions of this block)
                for jj in range(4):
                    jt = ib * 4 + jj
                    ps = gp.tile([P, PH], f32, tag="gp", name="vps")
                    for ct in range(CT):
                        nc.tensor.matmul(
                            ps, xts[ct][:, jj * P:(jj + 1) * P], wv_sb[:, ct, :],
                            start=(ct == 0), stop=(ct == CT - 1),
                        )
                    nc.vector.tensor_copy(
                        v_sb[:, jt, :].rearrange("p (g c) -> p g c", g=4)[:, :, 0:64],
                        ps.rearrange("p (g c) -> p g c", g=4),
                    )

            def attention(b, ib):
                q_sb, k_sb, v_sb, _ = get_state(b)
                isl = slice(ib * FB, (ib + 1) * FB)
                jmax = 4 * (ib + 1)
                avs_t = [
                    avp.tile([65, FB], f32, tag="av", name=f"av{g}")
                    for g in range(4)
                ]
                for jt in range(jmax):
                    sim = simp.tile([P, FB], f32, tag="sim", name="sim")
                    diag = jt >= 4 * ib
                    if diag:
                        # causal mask pre-added into PSUM (identity matmul)
                        nc.tensor.matmul(
                            sim, id_sb, tri_sb[:, jt - 4 * ib, :],
                            start=True, stop=True,
                        )
                    for g in range(4):
                        po = (g % 2) * 64
                        # stop=True after each group so the interleaved Exp
                        # read of the running cumsum is legal; stop is a
                        # no-op on hardware and accumulation continues
                        # (start=False keeps has_written semantics).
                        nc.tensor.matmul(
                            sim,
                            k_sb[po:po + 64, g // 2, jt * P:(jt + 1) * P],
                            q_sb[po:po + 64, g // 2, isl],
                            start=(g == 0 and not diag),
                            stop=True,
                            skip_group_check=(g > 0 or diag),
                        )
                        at = atp.tile([P, FB], f32r, tag="at", name="at")
                        nc.scalar.activation(at, sim, Exp)
                        nc.tensor.matmul(
                            avs_t[g],
                            v_sb[:, jt, g * 65:(g + 1) * 65],
                            at,
                            start=(jt == 0),
                            stop=(jt == jmax - 1),
                        )
                return avs_t

            def normalize(b, ib, avs_t):
                _, _, _, yin = get_state(b)
                isl = slice(ib * FB, (ib + 1) * FB)
                avs = avsp.tile([64, 4, FB], f32r, tag="avs", name="avs")
                for g in range(4):
                    rc = rcp.tile([65, FB], f32r, tag="rc", name="rc")
                    with nc.allow_low_precision(
                        reason="1/s rounded to f32r for the broadcast matmul"
                    ):
                        nc.vector.reciprocal(rc[64:65, :], avs_t[g][64:65, :])
                    bcps = gp.tile([64, FB], f32, tag="gp", name="bcps")
                    nc.tensor.matmul(
                        bcps, on_sb[64:65, :], rc[64:65, :], start=True, stop=True,
                    )
                    bc = bcp.tile([64, FB], f32, tag="bc", name="bc")
                    nc.vector.tensor_copy(bc, bcps)
                    nc.vector.tensor_mul(avs[:, g, :], avs_t[g][0:64, :], bc)
                # partition repack [64,(g)] -> [128,(gpair)] for the y matmul
                nc.sync.dma_start(
                    yin[0:64, :, isl],
                    avs.rearrange("p (gp two) f -> p two gp f", two=2)[:, 0],
                )
                nc.sync.dma_start(
                    yin[64:128, :, isl],
                    avs.rearrange("p (gp two) f -> p two gp f", two=2)[:, 1],
                )

            def yproj(b, ib):
                _, _, _, yin = get_state(b)
                isl = slice(ib * FB, (ib + 1) * FB)
                for ot in range(CT):
                    yp = gp.tile([P, FB], f32, tag="gp", name="yp")
                    for kc in range(2):
                        nc.tensor.matmul(
                            yp, wo_sb[:, kc, ot * P:(ot + 1) * P], yin[:, kc, isl],
                            start=(kc == 0), stop=(kc == 1),
                        )
                    ys = ysp.tile([P, FB], f32, tag="ys", name="ys")
                    nc.vector.tensor_copy(ys, yp)
                    nc.sync.dma_start(y_d[b, ot * P:(ot + 1) * P, isl], ys)

            # software pipeline: emit phase1 of step k+1 BEFORE yproj of
            # step k, so the PE always has dense independent work while the
            # DVE-heavy normalize chain runs (keeps HAM un-throttled and
            # avoids gp-pool false serialization).
            steps = [(b, ib) for b in range(B) for ib in range(NB)]
            phase1(*steps[0])
            for k, (b, ib) in enumerate(steps):
                avs_t = attention(b, ib)
                normalize(b, ib, avs_t)
                if k + 1 < len(steps):
                    phase1(*steps[k + 1])
                yproj(b, ib)
    if not nc.is_finalized():
        nc.finalize()
    return nc


def _host_inputs(x, Wq, Wkv, Wout):
    """Per-core input maps (head h on core h)."""
    import ml_dtypes

    tri = np.zeros((4, P, FB), np.float32)
    jj = np.arange(P)[:, None]
    ii = np.arange(FB)[None, :]
    for t in range(4):
        tri[t] = np.where(jj + t * P > ii, -1e30, 0.0)
    tri = tri.astype(ml_dtypes.bfloat16)
    ident = np.eye(P, dtype=np.float32).astype(ml_dtypes.bfloat16)

    x = np.ascontiguousarray(x, dtype=np.float32)
    Wk, Wv = Wkv[:CIN], Wkv[CIN:]
    in_maps = []
    for h in range(H):
        rows = (np.arange(G)[:, None] * (H * DH) + h * DH
                + np.arange(DH)[None, :]).reshape(-1)          # (g d) order
        in_maps.append({
            "x": x,
            "wqT": np.ascontiguousarray((Wq[rows] * SCALE).T, np.float32),
            "wkT": np.ascontiguousarray(Wk[rows].T, np.float32),
            "wvT": np.ascontiguousarray(Wv[rows].T, np.float32),
            "woT": np.ascontiguousarray(Wout[:, rows].T, np.float32),
            "tri": tri,
            "ident": ident,
            "onesr": np.ones((1, 64), np.float32),
            "vones": np.ones((P, JT, 4), np.float32),
        })
    return in_maps


def _install_profile_hook():
    """Register the NTFF profile hook that the image's antenv lacks, and
    keep profile artifacts local (no bucket upload)."""
    import sys
    import types
    import ctypes
    import contextlib

    if "antenv.axon_hooks" in sys.modules:
        return
    so_path = "/opt/axon/libaxon_pjrt.so"
    lib = ctypes.CDLL(so_path)
    if not hasattr(lib, "axon_start_nrt_profile"):
        raise RuntimeError("libaxon_pjrt.so lacks profiling symbols")
    lib.axon_start_nrt_profile.argtypes = [
        ctypes.POINTER(ctypes.c_int64), ctypes.c_size_t,
    ]
    lib.axon_start_nrt_profile.restype = ctypes.c_int64
    lib.axon_stop_nrt_profile.argtypes = [ctypes.c_char_p]
    lib.axon_stop_nrt_profile.restype = ctypes.c_int64

    @contextlib.contextmanager
    def _hook(output_dir, device_ids):
        import jax
        jax.devices()
        if device_ids:
            ids = (ctypes.c_int64 * len(device_ids))(*device_ids)
            rc = lib.axon_start_nrt_profile(ids, len(device_ids))
        else:
            rc = lib.axon_start_nrt_profile(None, 0)
        if rc != 0:
            raise RuntimeError(f"axon_start_nrt_profile rc={rc}")
        try:
            yield
        finally:
            n = lib.axon_stop_nrt_profile(str(output_dir).encode())
            print(f"profile: {n} file(s) written to {output_dir}")

    mod = types.ModuleType("antenv.axon_hooks")
    mod.get_axon_ntff_profile_hook = lambda: _hook
    mod.set_axon_ntff_profile_hook = lambda h: None
    sys.modules["antenv.axon_hooks"] = mod

    import concourse.bass_utils as bu
    bu.upload_artifacts = lambda tmpdir: tmpdir


def kernel(x, Wq, Wkv, Wout, b_out, _profile=False):
    import sys
    if "/opt/trn_rl_repo" not in sys.path:
        sys.path.insert(0, "/opt/trn_rl_repo")
    from concourse.bass_utils import run_bass_kernel_spmd
    if _profile:
        _install_profile_hook()

    if "nc" not in _cache:
        _cache["nc"] = _build_program()
    nc = _cache["nc"]
    in_maps = _host_inputs(
        np.asarray(x), np.asarray(Wq), np.asarray(Wkv), np.asarray(Wout)
    )
    kwargs = {}
    if _profile:
        import tempfile
        kwargs["tmpdir"] = tempfile.mkdtemp(prefix="bass_prof_")
    res = run_bass_kernel_spmd(
        nc, in_maps, core_ids=list(range(H)), trace=_profile, **kwargs
    )
    y = np.zeros((B, CIN, N), np.float64)
    for rmap in res.results:
        y += rmap["y"].astype(np.float64)
    y = y.astype(np.float32) + np.asarray(b_out)[None, :, None]
    if _profile:
        _cache["last_exec_time_ns"] = res.exec_time_ns
        _cache["last_profile_dir"] = kwargs.get("tmpdir")
        _cache["last_results"] = res
    return y


# revision 30
# speedup vs baseline: 1.2967x; 1.1226x over previous
"""Trainium2 Bass kernel for grouped cumulative-sim causal attention.

Reference computation (B=2, N=2048, G=4 groups, H=8 heads, DH=64):
  q/k/v = 1x1-conv projections of x [B, 2048, N]
  sim[b,g,h] = cumsum_over_g( (SCALE*q) @ k^T )   (the group-cumsum)
  out = softmax(causal(sim)) @ v ; y = Wout @ out + b_out

Sharding: one head h per NeuronCore (8 cores), both batches and all 4
groups local to the core (the cumsum couples g only). Each core computes
a partial y (its head's 256-channel contribution through Wout); the host
sums the 8 partials and adds b_out.

Device-side layout (everything transposed so the softmax j-axis lands on
PSUM partitions and attn comes out ready for the AV matmul):
  q,k  [dh=64(part,2 groups/tile), gpair, i/j]   (Wq pre-scaled by SCALE)
  sim_T[j(part), i]  accumulated in PSUM across g  -> group cumsum is free
  causal mask   pre-added into PSUM via identity-matmul of a -1e30 tile
  exp           ScalarE PSUM->SBUF
  AV            lhsT = v^T tile augmented with a ones column -> row 64 of
                the PSUM output accumulates the softmax denominator
  1/s broadcast K=1 matmul of the reciprocal row
  y             Wout^T-slice matmul, partials DMA'd out
Matmuls use float32r (full-rate fp32 on the PE at moving-dim>=256).
"""

import numpy as np

B, N = 2, 2048
G, H, DH = 4, 8, 64
CIN = 2048            # input channels  (= DIM*G)
PH = G * DH           # 256 inner channels per head
SCALE = DH ** -0.5
P = 128
FB = 512              # i-block width (fp32 moving-dim max)
NB = N // FB          # 4 i-blocks
CT = CIN // P         # 16 contraction tiles
JT = N // P           # 16 j-tiles

_cache = {}


def _build_program():
    import concourse.bass as bass  # noqa: F401
    import concourse.tile as tile
    from concourse import bacc, mybir

    f32 = mybir.dt.float32
    f32r = mybir.dt.float32r
    bf16 = mybir.dt.bfloat16
    Exp = mybir.ActivationFunctionType.Exp

    # Bacc (not raw Bass): its compile() splits multi-wait matmuls
    # (move_matmul_waits_to_ldweights / generate_event_semaphores) —
    # the S3_LW matmul instruction has a single hardware wait slot.
    nc = bacc.Bacc(None, target_bir_lowering=False)
    x_d = nc.dram_tensor("x", [B, CIN, N], f32r, kind="ExternalInput")
    wq_d = nc.dram_tensor("wqT", [CIN, PH], f32r, kind="ExternalInput")
    wk_d = nc.dram_tensor("wkT", [CIN, PH], f32r, kind="ExternalInput")
    wv_d = nc.dram_tensor("wvT", [CIN, PH], f32r, kind="ExternalInput")
    wo_d = nc.dram_tensor("woT", [PH, CIN], f32r, kind="ExternalInput")
    tri_d = nc.dram_tensor("tri", [4, P, FB], bf16, kind="ExternalInput")
    id_d = nc.dram_tensor("ident", [P, P], bf16, kind="ExternalInput")
    on_d = nc.dram_tensor("onesr", [1, 64], f32r, kind="ExternalInput")
    vo_d = nc.dram_tensor("vones", [P, JT, 4], f32r, kind="ExternalInput")
    y_d = nc.dram_tensor("y", [B, CIN, N], f32, kind="ExternalOutput")

    with tile.TileContext(nc) as tc:
        from contextlib import ExitStack

        with ExitStack() as ctx:
            consts = ctx.enter_context(tc.tile_pool(name="consts", bufs=1))
            big = ctx.enter_context(tc.tile_pool(name="big", bufs=1))
            xp = ctx.enter_context(tc.tile_pool(name="xp", bufs=18))
            atp = ctx.enter_context(tc.tile_pool(name="atp", bufs=5))
            avsp = ctx.enter_context(tc.tile_pool(name="avsp", bufs=1))
            rcp = ctx.enter_context(tc.tile_pool(name="rcp", bufs=2))
            bcp = ctx.enter_context(tc.tile_pool(name="bcp", bufs=2))
            ysp = ctx.enter_context(tc.tile_pool(name="ysp", bufs=3))
            simp = ctx.enter_context(tc.tile_pool(name="simp", bufs=2, space="PSUM"))
            avp = ctx.enter_context(tc.tile_pool(name="avp", bufs=4, space="PSUM"))
            gp = ctx.enter_context(tc.tile_pool(name="gp", bufs=2, space="PSUM"))

            # ---- static tensors ----
            wq_sb = consts.tile([P, CT, PH], f32r)
            wk_sb = consts.tile([P, CT, PH], f32r)
            wv_sb = consts.tile([P, CT, PH], f32r)
            wo_sb = consts.tile([P, 2, CIN], f32r)
            nc.sync.dma_start(wq_sb, wq_d[:, :].rearrange("(co ci) m -> ci co m", ci=P))
            nc.sync.dma_start(wk_sb, wk_d[:, :].rearrange("(co ci) m -> ci co m", ci=P))
            nc.sync.dma_start(wv_sb, wv_d[:, :].rearrange("(co ci) m -> ci co m", ci=P))
            nc.sync.dma_start(wo_sb, wo_d[:, :].rearrange("(ko ki) m -> ki ko m", ki=P))
            tri_sb = consts.tile([P, 4, FB], bf16)
            nc.sync.dma_start(tri_sb, tri_d[:, :, :].rearrange("t p f -> p t f"))
            id_sb = consts.tile([P, P], bf16)
            nc.sync.dma_start(id_sb, id_d[:, :])
            # ones row at partition 64 (matmul lhsT/rhs must share base
            # partition; the softmax denominator lives at partition 64)
            on_sb = consts.tile([65, 64], f32r)
            nc.sync.dma_start(on_sb[64:65, :], on_d[:, :])

            # per-batch state, allocated lazily by phase1(step) so the
            # software pipeline below controls allocation order
            state = {}

            def get_state(b):
                if b not in state:
                    q_sb = big.tile([P, 2, N], f32r, tag="q", name=f"q{b}")
                    k_sb = big.tile([P, 2, N], f32r, tag="k", name=f"k{b}")
                    v_sb = big.tile([P, JT, 4 * 65], f32r, tag="v", name=f"v{b}")
                    yin = big.tile([P, 2, N], f32r, tag="yin", name=f"yin{b}")
                    # ones column per group (softmax-denominator row of AV)
                    nc.sync.dma_start(
                        v_sb.rearrange("p t (g c) -> p t g c", g=4)[:, :, :, 64:65],
                        vo_d[:, :, :].rearrange("p t g -> p t g ()"),
                    )
                    state[b] = (q_sb, k_sb, v_sb, yin)
                return state[b]

            def phase1_units(b, ib):
                """Return a list of closures, one dense PE matmul chain each
                (4 q/k row-tiles + 4 v j-tiles); x DMAs are issued eagerly."""
                q_sb, k_sb, v_sb, _ = get_state(b)
                isl = slice(ib * FB, (ib + 1) * FB)
                xts = []
                for ct in range(CT):
                    xt = xp.tile([P, FB], f32r, tag="xt", name=f"xt{ct}")
                    nc.sync.dma_start(xt, x_d[b, ct * P:(ct + 1) * P, isl])
                    xts.append(xt)
                units = []

                def qk_unit(dest, wsb, m):
                    def emit():
                        ps = gp.tile([P, FB], f32, tag="gp", name="qkps")
                        for ct in range(CT):
                            nc.tensor.matmul(
                                ps, wsb[:, ct, m * P:(m + 1) * P], xts[ct],
                                start=(ct == 0), stop=(ct == CT - 1),
                            )
                        nc.vector.tensor_copy(dest[:, m, isl], ps)
                    return emit

                def v_unit(jj):
                    def emit():
                        jt = ib * 4 + jj
                        ps = gp.tile([P, PH], f32, tag="gp", name="vps")
                        for ct in range(CT):
                            nc.tensor.matmul(
                                ps, xts[ct][:, jj * P:(jj + 1) * P],
                                wv_sb[:, ct, :],
                                start=(ct == 0), stop=(ct == CT - 1),
                            )
                        nc.vector.tensor_copy(
                            v_sb[:, jt, :]
                            .rearrange("p (g c) -> p g c", g=4)[:, :, 0:64],
                            ps.rearrange("p (g c) -> p g c", g=4),
                        )
                    return emit

                for dest, wsb in ((q_sb, wq_sb), (k_sb, wk_sb)):
                    for m in range(2):
                        units.append(qk_unit(dest, wsb, m))
                for jj in range(4):
                    units.append(v_unit(jj))
                return units

            def yproj_units(b, ib):
                """One closure per output row-tile (2-matmul chain + copy + DMA)."""
                _, _, _, yin = get_state(b)
                isl = slice(ib * FB, (ib + 1) * FB)
                units = []

                def y_unit(ot):
                    def emit():
                        yp = gp.tile([P, FB], f32, tag="gp", name="yp")
                        for kc in range(2):
                            nc.tensor.matmul(
                                yp, wo_sb[:, kc, ot * P:(ot + 1) * P],
                                yin[:, kc, isl],
                                start=(kc == 0), stop=(kc == 1),
                            )
                        ys = ysp.tile([P, FB], f32, tag="ys", name="ys")
                        nc.vector.tensor_copy(ys, yp)
                        nc.sync.dma_start(y_d[b, ot * P:(ot + 1) * P, isl], ys)
                    return emit

                for ot in range(CT):
                    units.append(y_unit(ot))
                return units

            def attention(b, ib, filler):
                q_sb, k_sb, v_sb, _ = get_state(b)
                isl = slice(ib * FB, (ib + 1) * FB)
                jmax = 4 * (ib + 1)
                avs_t = [
                    avp.tile([65, FB], f32, tag="av", name=f"av{g}")
                    for g in range(4)
                ]
                nfill = len(filler)
                for jt in range(jmax):
                    # interleave dense independent matmul chains (next
                    # step's projections, previous step's y tiles) between
                    # the latency-bound cumsum chains so the PE stream
                    # stays dense and HAM stays un-throttled
                    take = (nfill * (jt + 1)) // jmax - (nfill * jt) // jmax
                    for _ in range(take):
                        filler.pop(0)()
                    sim = simp.tile([P, FB], f32, tag="sim", name="sim")
                    diag = jt >= 4 * ib
                    if diag:
                        # causal mask pre-added into PSUM (identity matmul)
                        nc.tensor.matmul(
                            sim, id_sb, tri_sb[:, jt - 4 * ib, :],
                            start=True, stop=True,
                        )
                    for g in range(4):
                        po = (g % 2) * 64
                        # stop=True after each group so the interleaved Exp
                        # read of the running cumsum is legal; stop is a
                        # no-op on hardware and accumulation continues
                        # (start=False keeps has_written semantics).
                        nc.tensor.matmul(
                            sim,
                            k_sb[po:po + 64, g // 2, jt * P:(jt + 1) * P],
                            q_sb[po:po + 64, g // 2, isl],
                            start=(g == 0 and not diag),
                            stop=True,
                            skip_group_check=(g > 0 or diag),
                        )
                        at = atp.tile([P, FB], f32r, tag="at", name="at")
                        nc.scalar.activation(at, sim, Exp)
                        nc.tensor.matmul(
                            avs_t[g],
                            v_sb[:, jt, g * 65:(g + 1) * 65],
                            at,
                            start=(jt == 0),
                            stop=(jt == jmax - 1),
                        )
                return avs_t

            def normalize(b, ib, avs_t):
                _, _, _, yin = get_state(b)
                isl = slice(ib * FB, (ib + 1) * FB)
                avs = avsp.tile([64, 4, FB], f32r, tag="avs", name="avs")
                for g in range(4):
                    rc = rcp.tile([65, FB], f32r, tag="rc", name="rc")
                    with nc.allow_low_precision(
                        reason="1/s rounded to f32r for the broadcast matmul"
                    ):
                        nc.vector.reciprocal(rc[64:65, :], avs_t[g][64:65, :])
                    bcps = gp.tile([64, FB], f32, tag="gp", name="bcps")
                    nc.tensor.matmul(
                        bcps, on_sb[64:65, :], rc[64:65, :], start=True, stop=True,
                    )
                    bc = bcp.tile([64, FB], f32, tag="bc", name="bc")
                    nc.vector.tensor_copy(bc, bcps)
                    nc.vector.tensor_mul(avs[:, g, :], avs_t[g][0:64, :], bc)
                # partition repack [64,(g)] -> [128,(gpair)] for the y matmul
                nc.sync.dma_start(
                    yin[0:64, :, isl],
                    avs.rearrange("p (gp two) f -> p two gp f", two=2)[:, 0],
                )
                nc.sync.dma_start(
                    yin[64:128, :, isl],
                    avs.rearrange("p (gp two) f -> p two gp f", two=2)[:, 1],
                )

            # software pipeline: attention(k) is emitted with the next
            # step's projection chains and the previous step's y tiles
            # interleaved between its cumsum chains.
            def interleave(a, bu):
                out = []
                la, lb = list(a), list(bu)
                while la or lb:
                    if la:
                        out.append(la.pop(0))
                    if lb:
                        out.append(lb.pop(0))
                return out

            steps = [(b, ib) for b in range(B) for ib in range(NB)]
            for u in phase1_units(*steps[0]):
                u()
            prev_yp = []
            for k, (b, ib) in enumerate(steps):
                nxt = phase1_units(*steps[k + 1]) if k + 1 < len(steps) else []
                filler = interleave(nxt, prev_yp)
                avs_t = attention(b, ib, filler)
                normalize(b, ib, avs_t)
                prev_yp = yproj_units(b, ib)
            for u in prev_yp:
                u()
    if not nc.is_finalized():
        nc.finalize()
    return nc


def _host_inputs(x, Wq, Wkv, Wout):
    """Per-core input maps (head h on core h)."""
    import ml_dtypes

    tri = np.zeros((4, P, FB), np.float32)
    jj = np.arange(P)[:, None]
    ii = np.arange(FB)[None, :]
    for t in range(4):
        tri[t] = np.where(jj + t * P > ii, -1e30, 0.0)
    tri = tri.astype(ml_dtypes.bfloat16)
    ident = np.eye(P, dtype=np.float32).astype(ml_dtypes.bfloat16)

    x = np.ascontiguousarray(x, dtype=np.float32)
    Wk, Wv = Wkv[:CIN], Wkv[CIN:]
    in_maps = []
    for h in range(H):
        rows = (np.arange(G)[:, None] * (H * DH) + h * DH
                + np.arange(DH)[None, :]).reshape(-1)          # (g d) order
        in_maps.append({
            "x": x,
            "wqT": np.ascontiguousarray((Wq[rows] * SCALE).T, np.float32),
            "wkT": np.ascontiguousarray(Wk[rows].T, np.float32),
            "wvT": np.ascontiguousarray(Wv[rows].T, np.float32),
            "woT": np.ascontiguousarray(Wout[:, rows].T, np.float32),
            "tri": tri,
            "ident": ident,
            "onesr": np.ones((1, 64), np.float32),
            "vones": np.ones((P, JT, 4), np.float32),
        })
    return in_maps


def _install_profile_hook():
    """Register the NTFF profile hook that the image's antenv lacks, and
    keep profile artifacts local (no bucket upload)."""
    import sys
    import types
    import ctypes
    import contextlib

    if "antenv.axon_hooks" in sys.modules:
        return
    so_path = "/opt/axon/libaxon_pjrt.so"
    lib = ctypes.CDLL(so_path)
    if not hasattr(lib, "axon_start_nrt_profile"):
        raise RuntimeError("libaxon_pjrt.so lacks profiling symbols")
    lib.axon_start_nrt_profile.argtypes = [
        ctypes.POINTER(ctypes.c_int64), ctypes.c_size_t,
    ]
    lib.axon_start_nrt_profile.restype = ctypes.c_int64
    lib.axon_stop_nrt_profile.argtypes = [ctypes.c_char_p]
    lib.axon_stop_nrt_profile.restype = ctypes.c_int64

    @contextlib.contextmanager
    def _hook(output_dir, device_ids):
        import jax
        jax.devices()
        if device_ids:
            ids = (ctypes.c_int64 * len(device_ids))(*device_ids)
            rc = lib.axon_start_nrt_profile(ids, len(device_ids))
        else:
            rc = lib.axon_start_nrt_profile(None, 0)
        if rc != 0:
            raise RuntimeError(f"axon_start_nrt_profile rc={rc}")
        try:
            yield
        finally:
            n = lib.axon_stop_nrt_profile(str(output_dir).encode())
            print(f"profile: {n} file(s) written to {output_dir}")

    mod = types.ModuleType("antenv.axon_hooks")
    mod.get_axon_ntff_profile_hook = lambda: _hook
    mod.set_axon_ntff_profile_hook = lambda h: None
    sys.modules["antenv.axon_hooks"] = mod

    import concourse.bass_utils as bu
    bu.upload_artifacts = lambda tmpdir: tmpdir


def kernel(x, Wq, Wkv, Wout, b_out, _profile=False):
    import sys
    if "/opt/trn_rl_repo" not in sys.path:
        sys.path.insert(0, "/opt/trn_rl_repo")
    from concourse.bass_utils import run_bass_kernel_spmd
    if _profile:
        _install_profile_hook()

    if "nc" not in _cache:
        _cache["nc"] = _build_program()
    nc = _cache["nc"]
    in_maps = _host_inputs(
        np.asarray(x), np.asarray(Wq), np.asarray(Wkv), np.asarray(Wout)
    )
    kwargs = {}
    if _profile:
        import tempfile
        kwargs["tmpdir"] = tempfile.mkdtemp(prefix="bass_prof_")
    res = run_bass_kernel_spmd(
        nc, in_maps, core_ids=list(range(H)), trace=_profile, **kwargs
    )
    y = np.zeros((B, CIN, N), np.float64)
    for rmap in res.results:
        y += rmap["y"].astype(np.float64)
    y = y.astype(np.float32) + np.asarray(b_out)[None, :, None]
    if _profile:
        _cache["last_exec_time_ns"] = res.exec_time_ns
        _cache["last_profile_dir"] = kwargs.get("tmpdir")
        _cache["last_results"] = res
    return y


# revision 31
# speedup vs baseline: 1.6301x; 1.2571x over previous
"""Trainium2 Bass kernel for grouped cumulative-sim causal attention.

Reference computation (B=2, N=2048, G=4 groups, H=8 heads, DH=64):
  q/k/v = 1x1-conv projections of x [B, 2048, N]
  sim[b,g,h] = cumsum_over_g( (SCALE*q) @ k^T )   (the group-cumsum)
  out = softmax(causal(sim)) @ v ; y = Wout @ out + b_out

Sharding: one head h per NeuronCore (8 cores), both batches and all 4
groups local to the core (the cumsum couples g only). Each core computes
a partial y (its head's 256-channel contribution through Wout); the host
sums the 8 partials and adds b_out.

Device-side layout (everything transposed so the softmax j-axis lands on
PSUM partitions and attn comes out ready for the AV matmul):
  q,k  [dh=64(part,2 groups/tile), gpair, i/j]   (Wq pre-scaled by SCALE)
  sim_T[j(part), i]  accumulated in PSUM across g  -> group cumsum is free
  causal mask   pre-added into PSUM via identity-matmul of a -1e30 tile
  exp           ScalarE PSUM->SBUF
  AV            lhsT = v^T tile augmented with a ones column -> row 64 of
                the PSUM output accumulates the softmax denominator
  1/s broadcast K=1 matmul of the reciprocal row
  y             Wout^T-slice matmul, partials DMA'd out
Matmuls use float32r (full-rate fp32 on the PE at moving-dim>=256).
"""

import numpy as np

B, N = 2, 2048
G, H, DH = 4, 8, 64
CIN = 2048            # input channels  (= DIM*G)
PH = G * DH           # 256 inner channels per head
SCALE = DH ** -0.5
P = 128
FB = 512              # i-block width (fp32 moving-dim max)
NB = N // FB          # 4 i-blocks
CT = CIN // P         # 16 contraction tiles
JT = N // P           # 16 j-tiles

_cache = {}


def _build_program():
    import concourse.bass as bass  # noqa: F401
    import concourse.tile as tile
    from concourse import bacc, mybir

    f32 = mybir.dt.float32
    f32r = mybir.dt.float32r
    bf16 = mybir.dt.bfloat16
    Exp = mybir.ActivationFunctionType.Exp

    # Bacc (not raw Bass): its compile() splits multi-wait matmuls
    # (move_matmul_waits_to_ldweights / generate_event_semaphores) —
    # the S3_LW matmul instruction has a single hardware wait slot.
    nc = bacc.Bacc(None, target_bir_lowering=False)
    x_d = nc.dram_tensor("x", [B, CIN, N], f32r, kind="ExternalInput")
    wq_d = nc.dram_tensor("wqT", [CIN, PH], f32r, kind="ExternalInput")
    wk_d = nc.dram_tensor("wkT", [CIN, PH], f32r, kind="ExternalInput")
    wv_d = nc.dram_tensor("wvT", [CIN, PH], f32r, kind="ExternalInput")
    wo_d = nc.dram_tensor("woT", [PH, CIN], f32r, kind="ExternalInput")
    tri_d = nc.dram_tensor("tri", [4, P, FB], bf16, kind="ExternalInput")
    id_d = nc.dram_tensor("ident", [P, P], bf16, kind="ExternalInput")
    on_d = nc.dram_tensor("onesr", [1, 64], f32r, kind="ExternalInput")
    vo_d = nc.dram_tensor("vones", [P, JT, 4], f32r, kind="ExternalInput")
    y_d = nc.dram_tensor("y", [B, CIN, N], f32, kind="ExternalOutput")

    with tile.TileContext(nc) as tc:
        from contextlib import ExitStack

        with ExitStack() as ctx:
            consts = ctx.enter_context(tc.tile_pool(name="consts", bufs=1))
            big = ctx.enter_context(tc.tile_pool(name="big", bufs=1))
            xp = ctx.enter_context(tc.tile_pool(name="xp", bufs=18))
            atp = ctx.enter_context(tc.tile_pool(name="atp", bufs=5))
            avsp = ctx.enter_context(tc.tile_pool(name="avsp", bufs=1))
            rcp = ctx.enter_context(tc.tile_pool(name="rcp", bufs=2))
            bcp = ctx.enter_context(tc.tile_pool(name="bcp", bufs=2))
            ysp = ctx.enter_context(tc.tile_pool(name="ysp", bufs=3))
            simp = ctx.enter_context(tc.tile_pool(name="simp", bufs=2, space="PSUM"))
            avp = ctx.enter_context(tc.tile_pool(name="avp", bufs=4, space="PSUM"))
            gp = ctx.enter_context(tc.tile_pool(name="gp", bufs=2, space="PSUM"))

            # ---- static tensors ----
            wq_sb = consts.tile([P, CT, PH], f32r)
            wk_sb = consts.tile([P, CT, PH], f32r)
            wv_sb = consts.tile([P, CT, PH], f32r)
            wo_sb = consts.tile([P, 2, CIN], f32r)
            nc.sync.dma_start(wq_sb, wq_d[:, :].rearrange("(co ci) m -> ci co m", ci=P))
            nc.sync.dma_start(wk_sb, wk_d[:, :].rearrange("(co ci) m -> ci co m", ci=P))
            nc.sync.dma_start(wv_sb, wv_d[:, :].rearrange("(co ci) m -> ci co m", ci=P))
            nc.sync.dma_start(wo_sb, wo_d[:, :].rearrange("(ko ki) m -> ki ko m", ki=P))
            tri_sb = consts.tile([P, 4, FB], bf16)
            nc.sync.dma_start(tri_sb, tri_d[:, :, :].rearrange("t p f -> p t f"))
            id_sb = consts.tile([P, P], bf16)
            nc.sync.dma_start(id_sb, id_d[:, :])
            # ones row at partition 64 (matmul lhsT/rhs must share base
            # partition; the softmax denominator lives at partition 64)
            on_sb = consts.tile([65, 64], f32r)
            nc.sync.dma_start(on_sb[64:65, :], on_d[:, :])

            # per-batch state, allocated lazily by phase1(step) so the
            # software pipeline below controls allocation order
            state = {}

            def get_state(b):
                if b not in state:
                    q_sb = big.tile([P, 2, N], f32r, tag="q", name=f"q{b}")
                    k_sb = big.tile([P, 2, N], f32r, tag="k", name=f"k{b}")
                    v_sb = big.tile([P, JT, 4 * 65], f32r, tag="v", name=f"v{b}")
                    yin = big.tile([P, 2, N], f32r, tag="yin", name=f"yin{b}")
                    # ones column per group (softmax-denominator row of AV)
                    nc.sync.dma_start(
                        v_sb.rearrange("p t (g c) -> p t g c", g=4)[:, :, :, 64:65],
                        vo_d[:, :, :].rearrange("p t g -> p t g ()"),
                    )
                    state[b] = (q_sb, k_sb, v_sb, yin)
                return state[b]

            def phase1_units(b, ib):
                """Return a list of closures, one dense PE matmul chain each
                (4 q/k row-tiles + 4 v j-tiles); x DMAs are issued eagerly."""
                q_sb, k_sb, v_sb, _ = get_state(b)
                isl = slice(ib * FB, (ib + 1) * FB)
                xts = []
                for ct in range(CT):
                    xt = xp.tile([P, FB], f32r, tag="xt", name=f"xt{ct}")
                    nc.sync.dma_start(xt, x_d[b, ct * P:(ct + 1) * P, isl])
                    xts.append(xt)
                units = []

                def qk_unit(dest, wsb, m):
                    def emit():
                        ps = gp.tile([P, FB], f32, tag="gp", name="qkps")
                        for ct in range(CT):
                            nc.tensor.matmul(
                                ps, wsb[:, ct, m * P:(m + 1) * P], xts[ct],
                                start=(ct == 0), stop=(ct == CT - 1),
                            )
                        nc.vector.tensor_copy(dest[:, m, isl], ps)
                    return emit

                def v_unit(jj):
                    def emit():
                        jt = ib * 4 + jj
                        ps = gp.tile([P, PH], f32, tag="gp", name="vps")
                        for ct in range(CT):
                            nc.tensor.matmul(
                                ps, xts[ct][:, jj * P:(jj + 1) * P],
                                wv_sb[:, ct, :],
                                start=(ct == 0), stop=(ct == CT - 1),
                            )
                        nc.vector.tensor_copy(
                            v_sb[:, jt, :]
                            .rearrange("p (g c) -> p g c", g=4)[:, :, 0:64],
                            ps.rearrange("p (g c) -> p g c", g=4),
                        )
                    return emit

                for dest, wsb in ((q_sb, wq_sb), (k_sb, wk_sb)):
                    for m in range(2):
                        units.append(qk_unit(dest, wsb, m))
                for jj in range(4):
                    units.append(v_unit(jj))
                return units

            def yproj_units(b, ib):
                """One closure per output row-tile (2-matmul chain + copy + DMA)."""
                _, _, _, yin = get_state(b)
                isl = slice(ib * FB, (ib + 1) * FB)
                units = []

                def y_unit(ot):
                    def emit():
                        yp = gp.tile([P, FB], f32, tag="gp", name="yp")
                        for kc in range(2):
                            nc.tensor.matmul(
                                yp, wo_sb[:, kc, ot * P:(ot + 1) * P],
                                yin[:, kc, isl],
                                start=(kc == 0), stop=(kc == 1),
                            )
                        ys = ysp.tile([P, FB], f32, tag="ys", name="ys")
                        nc.vector.tensor_copy(ys, yp)
                        nc.sync.dma_start(y_d[b, ot * P:(ot + 1) * P, isl], ys)
                    return emit

                for ot in range(CT):
                    units.append(y_unit(ot))
                return units

            def attention(b, ib, filler):
                q_sb, k_sb, v_sb, _ = get_state(b)
                isl = slice(ib * FB, (ib + 1) * FB)
                jmax = 4 * (ib + 1)
                avs_t = [
                    avp.tile([65, FB], f32, tag="av", name=f"av{g}")
                    for g in range(4)
                ]
                nfill = len(filler)
                for jt in range(jmax):
                    # interleave dense independent matmul chains (next
                    # step's projections, previous step's y tiles) between
                    # the attention chains so the PE stream stays dense
                    # and HAM stays un-throttled
                    take = (nfill * (jt + 1)) // jmax - (nfill * jt) // jmax
                    for _ in range(take):
                        filler.pop(0)()
                    jsl = slice(jt * P, (jt + 1) * P)
                    diag = jt >= 4 * ib
                    for g in range(4):
                        # cum_g = sum_{g'<=g} q_g'.k_g' as ONE stacked
                        # contraction over 64*(g+1) partitions (groups are
                        # partition-contiguous) -> the four cumulative sims
                        # are short independent chains, not a serial one.
                        sim = simp.tile([P, FB], f32, tag="sim", name="sim")
                        full, half = (g + 1) // 2, (g + 1) % 2
                        if diag:
                            # causal mask pre-added into PSUM
                            nc.tensor.matmul(
                                sim, id_sb, tri_sb[:, jt - 4 * ib, :],
                                start=True, stop=False,
                            )
                        for t in range(full):
                            nc.tensor.matmul(
                                sim, k_sb[:, t, jsl], q_sb[:, t, isl],
                                start=(not diag and t == 0),
                                stop=(t == full - 1 and not half),
                            )
                        if half:
                            nc.tensor.matmul(
                                sim, k_sb[0:64, full, jsl], q_sb[0:64, full, isl],
                                start=(not diag and full == 0),
                                stop=True,
                            )
                        at = atp.tile([P, FB], f32r, tag="at", name="at")
                        nc.scalar.activation(at, sim, Exp)
                        nc.tensor.matmul(
                            avs_t[g],
                            v_sb[:, jt, g * 65:(g + 1) * 65],
                            at,
                            start=(jt == 0),
                            stop=(jt == jmax - 1),
                        )
                return avs_t

            def normalize(b, ib, avs_t):
                _, _, _, yin = get_state(b)
                isl = slice(ib * FB, (ib + 1) * FB)
                avs = avsp.tile([64, 4, FB], f32r, tag="avs", name="avs")
                for g in range(4):
                    rc = rcp.tile([65, FB], f32r, tag="rc", name="rc")
                    with nc.allow_low_precision(
                        reason="1/s rounded to f32r for the broadcast matmul"
                    ):
                        nc.vector.reciprocal(rc[64:65, :], avs_t[g][64:65, :])
                    bcps = gp.tile([64, FB], f32, tag="gp", name="bcps")
                    nc.tensor.matmul(
                        bcps, on_sb[64:65, :], rc[64:65, :], start=True, stop=True,
                    )
                    bc = bcp.tile([64, FB], f32, tag="bc", name="bc")
                    nc.vector.tensor_copy(bc, bcps)
                    nc.vector.tensor_mul(avs[:, g, :], avs_t[g][0:64, :], bc)
                # partition repack [64,(g)] -> [128,(gpair)] for the y matmul
                nc.sync.dma_start(
                    yin[0:64, :, isl],
                    avs.rearrange("p (gp two) f -> p two gp f", two=2)[:, 0],
                )
                nc.sync.dma_start(
                    yin[64:128, :, isl],
                    avs.rearrange("p (gp two) f -> p two gp f", two=2)[:, 1],
                )

            # software pipeline: attention(k) is emitted with the next
            # step's projection chains and the previous step's y tiles
            # interleaved between its cumsum chains.
            def interleave(a, bu):
                out = []
                la, lb = list(a), list(bu)
                while la or lb:
                    if la:
                        out.append(la.pop(0))
                    if lb:
                        out.append(lb.pop(0))
                return out

            steps = [(b, ib) for b in range(B) for ib in range(NB)]
            for u in phase1_units(*steps[0]):
                u()
            prev_yp = []
            for k, (b, ib) in enumerate(steps):
                nxt = phase1_units(*steps[k + 1]) if k + 1 < len(steps) else []
                filler = interleave(nxt, prev_yp)
                avs_t = attention(b, ib, filler)
                normalize(b, ib, avs_t)
                prev_yp = yproj_units(b, ib)
            for u in prev_yp:
                u()
    if not nc.is_finalized():
        nc.finalize()
    return nc


def _host_inputs(x, Wq, Wkv, Wout):
    """Per-core input maps (head h on core h)."""
    import ml_dtypes

    tri = np.zeros((4, P, FB), np.float32)
    jj = np.arange(P)[:, None]
    ii = np.arange(FB)[None, :]
    for t in range(4):
        tri[t] = np.where(jj + t * P > ii, -1e30, 0.0)
    tri = tri.astype(ml_dtypes.bfloat16)
    ident = np.eye(P, dtype=np.float32).astype(ml_dtypes.bfloat16)

    x = np.ascontiguousarray(x, dtype=np.float32)
    Wk, Wv = Wkv[:CIN], Wkv[CIN:]
    in_maps = []
    for h in range(H):
        rows = (np.arange(G)[:, None] * (H * DH) + h * DH
                + np.arange(DH)[None, :]).reshape(-1)          # (g d) order
        in_maps.append({
            "x": x,
            "wqT": np.ascontiguousarray((Wq[rows] * SCALE).T, np.float32),
            "wkT": np.ascontiguousarray(Wk[rows].T, np.float32),
            "wvT": np.ascontiguousarray(Wv[rows].T, np.float32),
            "woT": np.ascontiguousarray(Wout[:, rows].T, np.float32),
            "tri": tri,
            "ident": ident,
            "onesr": np.ones((1, 64), np.float32),
            "vones": np.ones((P, JT, 4), np.float32),
        })
    return in_maps


def _install_profile_hook():
    """Register the NTFF profile hook that the image's antenv lacks, and
    keep profile artifacts local (no bucket upload)."""
    import sys
    import types
    import ctypes
    import contextlib

    if "antenv.axon_hooks" in sys.modules:
        return
    so_path = "/opt/axon/libaxon_pjrt.so"
    lib = ctypes.CDLL(so_path)
    if not hasattr(lib, "axon_start_nrt_profile"):
        raise RuntimeError("libaxon_pjrt.so lacks profiling symbols")
    lib.axon_start_nrt_profile.argtypes = [
        ctypes.POINTER(ctypes.c_int64), ctypes.c_size_t,
    ]
    lib.axon_start_nrt_profile.restype = ctypes.c_int64
    lib.axon_stop_nrt_profile.argtypes = [ctypes.c_char_p]
    lib.axon_stop_nrt_profile.restype = ctypes.c_int64

    @contextlib.contextmanager
    def _hook(output_dir, device_ids):
        import jax
        jax.devices()
        if device_ids:
            ids = (ctypes.c_int64 * len(device_ids))(*device_ids)
            rc = lib.axon_start_nrt_profile(ids, len(device_ids))
        else:
            rc = lib.axon_start_nrt_profile(None, 0)
        if rc != 0:
            raise RuntimeError(f"axon_start_nrt_profile rc={rc}")
        try:
            yield
        finally:
            n = lib.axon_stop_nrt_profile(str(output_dir).encode())
            print(f"profile: {n} file(s) written to {output_dir}")

    mod = types.ModuleType("antenv.axon_hooks")
    mod.get_axon_ntff_profile_hook = lambda: _hook
    mod.set_axon_ntff_profile_hook = lambda h: None
    sys.modules["antenv.axon_hooks"] = mod

    import concourse.bass_utils as bu
    bu.upload_artifacts = lambda tmpdir: tmpdir


def kernel(x, Wq, Wkv, Wout, b_out, _profile=False):
    import sys
    if "/opt/trn_rl_repo" not in sys.path:
        sys.path.insert(0, "/opt/trn_rl_repo")
    from concourse.bass_utils import run_bass_kernel_spmd
    if _profile:
        _install_profile_hook()

    if "nc" not in _cache:
        _cache["nc"] = _build_program()
    nc = _cache["nc"]
    in_maps = _host_inputs(
        np.asarray(x), np.asarray(Wq), np.asarray(Wkv), np.asarray(Wout)
    )
    kwargs = {}
    if _profile:
        import tempfile
        kwargs["tmpdir"] = tempfile.mkdtemp(prefix="bass_prof_")
    res = run_bass_kernel_spmd(
        nc, in_maps, core_ids=list(range(H)), trace=_profile, **kwargs
    )
    y = np.zeros((B, CIN, N), np.float64)
    for rmap in res.results:
        y += rmap["y"].astype(np.float64)
    y = y.astype(np.float32) + np.asarray(b_out)[None, :, None]
    if _profile:
        _cache["last_exec_time_ns"] = res.exec_time_ns
        _cache["last_profile_dir"] = kwargs.get("tmpdir")
        _cache["last_results"] = res
    return y
